# revision 30
# baseline (speedup 1.0000x reference)
"""Trainium2 Bass kernel for nn_AttnReducerRNN (attention-reducer GRU decoder step).

Self-contained: takes FULL inputs, shards across 8 NeuronCores internally,
returns FULL outputs (out [1,1], h_new [1,1,1024], attn_weights [1,16384]).

Decomposition (per core c of 8):
  - embedding row gathered host-side (only 4KB of the 205MB table is live).
  - q = attn_w @ [emb;h0]: M-sharded (128 rows/core) + AllGather.
  - encoder_outputs row-sharded (2048 rows/core), streamed to SBUF once;
    norms on ScalarE (square+accum), dots on VectorE (fused mul+reduce),
    exp on ScalarE (cosine sims are in [-1,1] -> no max subtraction needed),
    weighted sum S = sum_i w_i e_i on TensorE (f32r moving-operand matmuls).
  - One AllReduce carries [S | gh_partial | total]: gh = w_hh @ h0 is
    K-sharded and computed early, hiding its cost under the DVE phase.
  - comb layer M-sharded (x2_c column stays local), gi = w_ih @ x2
    K-sharded + second AllReduce; GRU gates + fc2 + out replicated (tiny).
"""
import os
import numpy as np

NC = 8
P = 128
H = 1024
L = 16384
LS = L // NC          # 2048 enc rows per core
HS = H // NC          # 128
T_ENC = LS // P       # 16 enc tiles per core
KT = 2048 // P        # 16 k-tiles of the concat [emb;h0] vector
MC = 3 * H // P       # 24 m-chunks of the 3072 gate vector
EPS = 1e-8

_STATE: dict = {}
USE_F32R = os.environ.get("KERNEL_F32R", "1") == "1"
PHASE = int(os.environ.get("KERNEL_PHASE", "5"))
SUB = os.environ.get("KERNEL_SUB", "d")  # a=squares, b=+ttr, c=+gh, d=+softmax


def _build_nc():
    import concourse.bass as bass
    import concourse.bacc as bacc
    import concourse.mybir as mybir
    import concourse.tile as tile
    from contextlib import ExitStack

    f32 = mybir.dt.float32
    f32r = mybir.dt.float32r if USE_F32R else mybir.dt.float32
    AF = mybir.ActivationFunctionType
    ALU = mybir.AluOpType
    RG = [list(range(NC))]

    nc = bacc.Bacc("TRN2", target_bir_lowering=False, debug=False, num_devices=NC)

    # ---------------- I/O ----------------
    enc_d = nc.dram_tensor("enc", [LS, H], f32r, kind="ExternalInput")
    attn_wT_d = nc.dram_tensor("attn_wT", [2048, P], f32, kind="ExternalInput")
    attn_b_d = nc.dram_tensor("attn_b", [P], f32, kind="ExternalInput")
    x_vec_d = nc.dram_tensor("x_vec", [2048], f32, kind="ExternalInput")
    h0c_d = nc.dram_tensor("h0c", [P], f32, kind="ExternalInput")
    comb_wT_d = nc.dram_tensor("comb_wT", [2048, P], f32, kind="ExternalInput")
    comb_b_d = nc.dram_tensor("comb_b", [P], f32, kind="ExternalInput")
    w_ihT_d = nc.dram_tensor("w_ihT", [P, 3 * H], f32, kind="ExternalInput")
    w_hhT_d = nc.dram_tensor("w_hhT", [P, 3 * H], f32, kind="ExternalInput")
    b_ih_d = nc.dram_tensor("b_ih_col", [P, MC], f32, kind="ExternalInput")
    b_hh_d = nc.dram_tensor("b_hh_col", [P, MC], f32, kind="ExternalInput")
    fc2_wT_d = nc.dram_tensor("fc2_wT", [H, 64], f32, kind="ExternalInput")
    fc2_b_d = nc.dram_tensor("fc2_b", [64], f32, kind="ExternalInput")
    out_wT_d = nc.dram_tensor("out_wT", [64], f32, kind="ExternalInput")
    out_b_d = nc.dram_tensor("out_b", [1], f32, kind="ExternalInput")

    out_s_d = nc.dram_tensor("out_s", [1, 1], f32, kind="ExternalOutput")
    h_new_d = nc.dram_tensor("h_new_out", [H], f32, kind="ExternalOutput")
    attn_part_d = nc.dram_tensor("attn_part", [LS], f32, kind="ExternalOutput")

    # collective bounce buffers (internal DRAM; outputs Shared)
    qag_in = nc.dram_tensor("qag_in", [P], f32)
    qag_out = nc.dram_tensor("qag_out", [H], f32, addr_space="Shared")
    AR1 = 4104  # S[0:1024] | gh[1024:4096] | total[4096] | pad[4097:4104]
    ar1_in = nc.dram_tensor("ar1_in", [AR1], f32)
    ar1_out = nc.dram_tensor("ar1_out", [AR1], f32, addr_space="Shared")
    ar2_in = nc.dram_tensor("ar2_in", [3 * H], f32)
    ar2_out = nc.dram_tensor("ar2_out", [3 * H], f32, addr_space="Shared")

    with tile.TileContext(nc) as tc:
        ctx = ExitStack()
        pool = ctx.enter_context(tc.tile_pool(name="main", bufs=1))
        scratch = ctx.enter_context(tc.tile_pool(name="scratch", bufs=2))

        # -------- constants --------
        ones_row = pool.tile([1, P], f32, name="ones_row")
        nc.vector.memset(ones_row[:], 1.0)
        ones_col = pool.tile([P, 1], f32, name="ones_col")
        nc.vector.memset(ones_col[:], 1.0)
        zeros8 = pool.tile([1, 8], f32, name="zeros8")
        nc.vector.memset(zeros8[:], 0.0)
        nc.scalar.dma_start(
            out=ar1_in.ap()[4097:4104].rearrange("(o k) -> o k", o=1),
            in_=zeros8[:, 0:7],
        )

        # -------- input DMAs --------
        x_col = pool.tile([P, KT], f32, name="x_col")
        nc.sync.dma_start(out=x_col[:], in_=x_vec_d.ap().rearrange("(t p) -> p t", p=P))
        attn_b_sb = pool.tile([P, 1], f32, name="attn_b_sb")
        nc.sync.dma_start(out=attn_b_sb[:], in_=attn_b_d.ap().rearrange("(p o) -> p o", o=1))
        h0c_col = pool.tile([P, 1], f32, name="h0c_col")
        nc.sync.dma_start(out=h0c_col[:], in_=h0c_d.ap().rearrange("(p o) -> p o", o=1))

        attn_wT_sb = pool.tile([P, KT, P], f32, name="attn_wT_sb")
        nc.sync.dma_start(
            out=attn_wT_sb[:], in_=attn_wT_d.ap().rearrange("(t p) m -> p t m", p=P)
        )

        enc_sb = []
        for t in range(T_ENC):
            et = pool.tile([P, H], f32r, name=f"enc{t}")
            nc.sync.dma_start(out=et[:], in_=enc_d.ap()[t * P:(t + 1) * P, :])
            enc_sb.append(et)

        w_hhT_sb = pool.tile([P, MC, P], f32, name="w_hhT_sb")
        nc.sync.dma_start(
            out=w_hhT_sb[:], in_=w_hhT_d.ap().rearrange("k (mc m) -> k mc m", m=P)
        )
        comb_wT_sb = pool.tile([P, KT, P], f32, name="comb_wT_sb")
        nc.sync.dma_start(
            out=comb_wT_sb[:], in_=comb_wT_d.ap().rearrange("(t p) m -> p t m", p=P)
        )
        comb_b_sb = pool.tile([P, 1], f32, name="comb_b_sb")
        nc.sync.dma_start(out=comb_b_sb[:], in_=comb_b_d.ap().rearrange("(p o) -> p o", o=1))
        w_ihT_sb = pool.tile([P, MC, P], f32, name="w_ihT_sb")
        nc.sync.dma_start(
            out=w_ihT_sb[:], in_=w_ihT_d.ap().rearrange("k (mc m) -> k mc m", m=P)
        )
        b_ih_sb = pool.tile([P, MC], f32, name="b_ih_sb")
        nc.sync.dma_start(out=b_ih_sb[:], in_=b_ih_d.ap())
        b_hh_sb = pool.tile([P, MC], f32, name="b_hh_sb")
        nc.sync.dma_start(out=b_hh_sb[:], in_=b_hh_d.ap())
        fc2_wT_sb = pool.tile([P, 8, 64], f32, name="fc2_wT_sb")
        nc.sync.dma_start(
            out=fc2_wT_sb[:], in_=fc2_wT_d.ap().rearrange("(t p) m -> p t m", p=P)
        )
        fc2_b_sb = pool.tile([64, 1], f32, name="fc2_b_sb")
        nc.sync.dma_start(out=fc2_b_sb[:], in_=fc2_b_d.ap().rearrange("(p o) -> p o", o=1))
        out_wT_sb = pool.tile([64, 1], f32, name="out_wT_sb")
        nc.sync.dma_start(out=out_wT_sb[:], in_=out_wT_d.ap().rearrange("(p o) -> p o", o=1))
        out_b_sb = pool.tile([1, 1], f32, name="out_b_sb")
        nc.sync.dma_start(out=out_b_sb[:], in_=out_b_d.ap().rearrange("(o k) -> o k", o=1))

        # -------- phase 1: q shard + AllGather + broadcast --------
        with nc.named_scope("q_phase"):
            q_psum, q_psum_free = tc.tile([P, 1], f32, space="PSUM", name="q_psum")
            for t in range(KT):
                nc.tensor.matmul(
                    q_psum[:], attn_wT_sb[:, t, :], x_col[:, t:t + 1],
                    start=(t == 0), stop=(t == KT - 1),
                )
            q_sb = pool.tile([P, 1], f32, name="q_sb")
            nc.scalar.activation(q_sb[:], q_psum[:], AF.Identity, bias=attn_b_sb[:])
            q_psum_free()
            nc.scalar.dma_start(
                out=qag_in.ap().rearrange("(p o) -> p o", o=1), in_=q_sb[:]
            )
            nc.gpsimd.collective_compute(
                "AllGather", ALU.bypass, replica_groups=RG,
                ins=[qag_in.ap()], outs=[qag_out.ap()],
            )
            # broadcast q across all 128 partitions with a stride-0 DMA read
            q_bcast = pool.tile([P, H], f32, name="q_bcast")
            nc.scalar.dma_start(
                out=q_bcast[:],
                in_=qag_out.ap().partition_broadcast(P),
            )
            # |q|^2 identical per partition via square+accumulate over q_bcast
            qsq_scr = scratch.tile([P, H], f32, tag="sq", name="qsq_scr")
            qsq_acc = pool.tile([P, 1], f32, name="qsq_acc")
            nc.scalar.activation(qsq_scr[:], q_bcast[:], AF.Square, accum_out=qsq_acc[:])
            qn = pool.tile([P, 1], f32, name="qn")
            nc.scalar.activation(qn[:], qsq_acc[:], AF.Sqrt)
            qn2 = pool.tile([P, 1], f32, name="qn2")
            nc.vector.tensor_scalar_max(qn2[:], qn[:], EPS)
            inv_qn = pool.tile([P, 1], f32, name="inv_qn")
            nc.vector.reciprocal(inv_qn[:], qn2[:])

        if PHASE >= 2:
            # -------- phase 2: per-tile norms (ACT) + dots (DVE) --------
            normsq = pool.tile([P, T_ENC], f32, name="normsq")
            dots = pool.tile([P, T_ENC], f32, name="dots")
            with nc.named_scope("norms_dots"):
                for t in range(T_ENC):
                    sq_t = scratch.tile([P, H], f32, tag="sq", name=f"sq{t}")
                    nc.scalar.activation(
                        sq_t[:], enc_sb[t].bitcast(f32), AF.Square,
                        accum_out=normsq[:, t:t + 1],
                    )
                    if SUB >= "b":
                        prod_t = scratch.tile([P, H], f32, tag="prod", name=f"prod{t}")
                        nc.vector.scalar_tensor_tensor(
                            out=prod_t[:], in0=enc_sb[t].bitcast(f32), scalar=1.0,
                            in1=q_bcast[:], op0=ALU.mult, op1=ALU.mult,
                            accum_out=dots[:, t:t + 1],
                        )
                if SUB < "b":
                    nc.vector.memset(dots[:], 1.0)

            # -------- phase 2b: gh partial on PE (independent of attention) --
            with nc.named_scope("gh_partial") if SUB >= "c" else nc.named_scope("gh_skip"):
              if SUB >= "c":
                gh_psum, gh_psum_free = tc.tile([P, MC], f32, space="PSUM", name="gh_psum")
                for mc in range(MC):
                    nc.tensor.matmul(
                        gh_psum[:, mc:mc + 1], w_hhT_sb[:, mc, :], h0c_col[:],
                        start=(mc == 0), stop=(mc == MC - 1),
                    )
                gh_stage = pool.tile([P, MC], f32, name="gh_stage")
                nc.scalar.copy(gh_stage[:], gh_psum[:])
                gh_psum_free()
                nc.scalar.dma_start(
                    out=ar1_in.ap()[1024:4096].rearrange("(mc p) -> p mc", p=P),
                    in_=gh_stage[:],
                )

            # -------- phase 3: softmax numerators --------
            with nc.named_scope("softmax"):
              if SUB >= "d":
                en = pool.tile([P, T_ENC], f32, name="en")
                nc.scalar.activation(en[:], normsq[:], AF.Sqrt)
                en2 = pool.tile([P, T_ENC], f32, name="en2")
                nc.vector.tensor_scalar_max(en2[:], en[:], EPS)
                inv_en = pool.tile([P, T_ENC], f32, name="inv_en")
                nc.vector.reciprocal(inv_en[:], en2[:])
                sims_n = pool.tile([P, T_ENC], f32, name="sims_n")
                nc.vector.tensor_mul(sims_n[:], dots[:], inv_en[:])
                w_unnorm = pool.tile([P, T_ENC], f32r, name="w_unnorm")
                sumexp_part = pool.tile([P, 1], f32, name="sumexp_part")
                nc.scalar.activation(
                    w_unnorm[:], sims_n[:], AF.Exp, scale=inv_qn[:],
                    accum_out=sumexp_part[:],
                )

        if PHASE >= 3:
            # -------- phase 4: S = sum_i w_i e_i (f32r), total, AllReduce ----
            with nc.named_scope("passC"):
                S_psum, S_psum_free = tc.tile([1, H], f32, space="PSUM", name="S_psum")
                for t in range(T_ENC):
                    for h in range(2):
                        nc.tensor.matmul(
                            S_psum[:, h * 512:(h + 1) * 512],
                            w_unnorm[:, t:t + 1],
                            enc_sb[t][:, h * 512:(h + 1) * 512],
                            start=(t == 0), stop=(t == T_ENC - 1),
                        )
                S_stage = pool.tile([1, H], f32, name="S_stage")
                nc.scalar.copy(S_stage[:], S_psum[:])
                S_psum_free()
                tot_psum, tot_psum_free = tc.tile([1, 1], f32, space="PSUM", name="tot_psum")
                nc.tensor.matmul(tot_psum[:], sumexp_part[:], ones_col[:], start=True, stop=True)
                tot_stage = pool.tile([1, 1], f32, name="tot_stage")
                nc.scalar.copy(tot_stage[:], tot_psum[:])
                tot_psum_free()
                nc.scalar.dma_start(
                    out=ar1_in.ap()[0:1024].rearrange("(o k) -> o k", o=1), in_=S_stage[:]
                )
                nc.scalar.dma_start(
                    out=ar1_in.ap()[4096:4097].rearrange("(o k) -> o k", o=1),
                    in_=tot_stage[:],
                )
                nc.gpsimd.collective_compute(
                    "AllReduce", ALU.add, replica_groups=RG,
                    ins=[ar1_in.ap()], outs=[ar1_out.ap()],
                )

        if PHASE >= 4:
            # -------- phase 5: attn_applied, attn_weights out, comb layer ----
            with nc.named_scope("comb"):
                S_col = pool.tile([P, 8], f32, name="S_col")
                nc.scalar.dma_start(
                    out=S_col[:], in_=ar1_out.ap()[0:1024].rearrange("(kc p) -> p kc", p=P)
                )
                gh_sb = pool.tile([P, MC], f32, name="gh_sb")
                nc.scalar.dma_start(
                    out=gh_sb[:], in_=ar1_out.ap()[1024:4096].rearrange("(mc p) -> p mc", p=P)
                )
                tot_sb = pool.tile([1, 1], f32, name="tot_sb")
                nc.scalar.dma_start(
                    out=tot_sb[:], in_=ar1_out.ap()[4096:4097].rearrange("(o k) -> o k", o=1)
                )
                inv_tot = pool.tile([1, 1], f32, name="inv_tot")
                nc.vector.reciprocal(inv_tot[:], tot_sb[:])
                invt_psum, invt_psum_free = tc.tile([P, 1], f32, space="PSUM", name="invt_psum")
                nc.tensor.matmul(invt_psum[:], ones_row[:], inv_tot[:], start=True, stop=True)
                inv_tot_b = pool.tile([P, 1], f32, name="inv_tot_b")
                nc.scalar.copy(inv_tot_b[:], invt_psum[:])
                invt_psum_free()

                attn_sb = pool.tile([P, T_ENC], f32, name="attn_sb")
                nc.vector.tensor_scalar_mul(attn_sb[:], w_unnorm.bitcast(f32), inv_tot_b[:])
                nc.sync.dma_start(
                    out=attn_part_d.ap().rearrange("(t p) -> p t", p=P), in_=attn_sb[:]
                )

                y_att = pool.tile([P, 8], f32, name="y_att")
                nc.vector.tensor_scalar_mul(y_att[:], S_col[:], inv_tot_b[:])

                x2_psum, x2_psum_free = tc.tile([P, 1], f32, space="PSUM", name="x2_psum")
                for t in range(KT):
                    rhs = x_col[:, t:t + 1] if t < 8 else y_att[:, t - 8:t - 7]
                    nc.tensor.matmul(
                        x2_psum[:], comb_wT_sb[:, t, :], rhs,
                        start=(t == 0), stop=(t == KT - 1),
                    )
                x2_sb = pool.tile([P, 1], f32, name="x2_sb")
                nc.scalar.activation(x2_sb[:], x2_psum[:], AF.Relu, bias=comb_b_sb[:])
                x2_psum_free()

            # -------- phase 6: gi partial + AllReduce --------
            with nc.named_scope("gi"):
                gi_psum, gi_psum_free = tc.tile([P, MC], f32, space="PSUM", name="gi_psum")
                for mc in range(MC):
                    nc.tensor.matmul(
                        gi_psum[:, mc:mc + 1], w_ihT_sb[:, mc, :], x2_sb[:],
                        start=(mc == 0), stop=(mc == MC - 1),
                    )
                gi_stage = pool.tile([P, MC], f32, name="gi_stage")
                nc.scalar.copy(gi_stage[:], gi_psum[:])
                gi_psum_free()
                nc.scalar.dma_start(
                    out=ar2_in.ap().rearrange("(mc p) -> p mc", p=P), in_=gi_stage[:]
                )
                nc.gpsimd.collective_compute(
                    "AllReduce", ALU.add, replica_groups=RG,
                    ins=[ar2_in.ap()], outs=[ar2_out.ap()],
                )
                gi_sb = pool.tile([P, MC], f32, name="gi_sb")
                nc.scalar.dma_start(
                    out=gi_sb[:], in_=ar2_out.ap().rearrange("(mc p) -> p mc", p=P)
                )

        if PHASE >= 5:
            # -------- phase 7: gates, h_new, fc2, out --------
            with nc.named_scope("tail"):
                gi_b = pool.tile([P, MC], f32, name="gi_b")
                nc.vector.tensor_add(gi_b[:], gi_sb[:], b_ih_sb[:])
                gh_b = pool.tile([P, MC], f32, name="gh_b")
                nc.vector.tensor_add(gh_b[:], gh_sb[:], b_hh_sb[:])
                rz_sum = pool.tile([P, 16], f32, name="rz_sum")
                nc.vector.tensor_add(rz_sum[:], gi_b[:, 0:16], gh_b[:, 0:16])
                rz = pool.tile([P, 16], f32, name="rz")
                nc.scalar.activation(rz[:], rz_sum[:], AF.Sigmoid)
                rn_tmp = pool.tile([P, 8], f32, name="rn_tmp")
                nc.vector.tensor_mul(rn_tmp[:], rz[:, 0:8], gh_b[:, 16:24])
                n_in = pool.tile([P, 8], f32, name="n_in")
                nc.vector.tensor_add(n_in[:], rn_tmp[:], gi_b[:, 16:24])
                n_t = pool.tile([P, 8], f32, name="n_t")
                nc.scalar.activation(n_t[:], n_in[:], AF.Tanh)
                hmn = pool.tile([P, 8], f32, name="hmn")
                nc.vector.tensor_sub(hmn[:], x_col[:, 8:16], n_t[:])
                zt = pool.tile([P, 8], f32, name="zt")
                nc.vector.tensor_mul(zt[:], rz[:, 8:16], hmn[:])
                h_new_sb = pool.tile([P, 8], f32, name="h_new_sb")
                nc.vector.tensor_add(h_new_sb[:], n_t[:], zt[:])
                nc.sync.dma_start(
                    out=h_new_d.ap().rearrange("(t p) -> p t", p=P), in_=h_new_sb[:]
                )

                fc2_psum, fc2_psum_free = tc.tile([64, 1], f32, space="PSUM", name="fc2_psum")
                for kc in range(8):
                    nc.tensor.matmul(
                        fc2_psum[:], fc2_wT_sb[:, kc, :], h_new_sb[:, kc:kc + 1],
                        start=(kc == 0), stop=(kc == 7),
                    )
                relu64 = pool.tile([64, 1], f32, name="relu64")
                nc.scalar.activation(relu64[:], fc2_psum[:], AF.Relu, bias=fc2_b_sb[:])
                fc2_psum_free()
                outp, outp_free = tc.tile([1, 1], f32, space="PSUM", name="outp")
                nc.tensor.matmul(outp[:], relu64[:], out_wT_sb[:], start=True, stop=True)
                out_sb = pool.tile([1, 1], f32, name="out_sb")
                nc.scalar.activation(out_sb[:], outp[:], AF.Identity, bias=out_b_sb[:])
                outp_free()
                nc.sync.dma_start(out=out_s_d.ap(), in_=out_sb[:])

        # -------- bisection stubs for outputs not produced at this PHASE ----
        if PHASE < 4:
            nc.sync.dma_start(
                out=attn_part_d.ap().rearrange("(t p) -> p t", p=P), in_=x_col[:]
            )
        if PHASE < 5:
            nc.sync.dma_start(
                out=h_new_d.ap().rearrange("(t p) -> p t", p=P), in_=x_col[:, 8:16]
            )
            nc.sync.dma_start(out=out_s_d.ap(), in_=attn_b_sb[0:1, 0:1])

        ctx.close()

    nc.compile()
    return nc


def _shard_inputs(token, hidden, encoder_outputs, emb, attn_w, attn_b, comb_w,
                  comb_b, w_ih, w_hh, b_ih, b_hh, fc2_w, fc2_b, out_w, out_b):
    f = lambda a: np.ascontiguousarray(np.asarray(a, dtype=np.float32))
    token = np.asarray(token)
    emb = np.asarray(emb)
    embedded = f(emb[int(token[0, 0])])                    # [1024]
    h0 = f(np.asarray(hidden)[0, 0])                       # [1024]
    x_vec = np.concatenate([embedded, h0])                 # [2048]
    enc = f(encoder_outputs)
    attn_w = f(attn_w); attn_b = f(attn_b)
    comb_w = f(comb_w); comb_b = f(comb_b)
    w_ih = f(w_ih); w_hh = f(w_hh); b_ih = f(b_ih); b_hh = f(b_hh)
    fc2_wT = np.ascontiguousarray(f(fc2_w).T)              # [1024, 64]
    fc2_b = f(fc2_b)
    out_wT = f(out_w)[0]                                   # [64]
    out_b = f(out_b)
    b_ih_col = np.ascontiguousarray(b_ih.reshape(MC, P).T)  # [128, 24]
    b_hh_col = np.ascontiguousarray(b_hh.reshape(MC, P).T)

    in_maps = []
    for c in range(NC):
        sl = slice(c * HS, (c + 1) * HS)
        in_maps.append({
            "enc": np.ascontiguousarray(enc[c * LS:(c + 1) * LS]),
            "attn_wT": np.ascontiguousarray(attn_w[sl, :].T),
            "attn_b": np.ascontiguousarray(attn_b[sl]),
            "x_vec": x_vec,
            "h0c": np.ascontiguousarray(h0[sl]),
            "comb_wT": np.ascontiguousarray(comb_w[sl, :].T),
            "comb_b": np.ascontiguousarray(comb_b[sl]),
            "w_ihT": np.ascontiguousarray(w_ih[:, sl].T),
            "w_hhT": np.ascontiguousarray(w_hh[:, sl].T),
            "b_ih_col": b_ih_col,
            "b_hh_col": b_hh_col,
            "fc2_wT": fc2_wT,
            "fc2_b": fc2_b,
            "out_wT": out_wT,
            "out_b": out_b,
        })
    return in_maps


def kernel(**inputs):
    from concourse.bass_utils import run_bass_kernel_spmd

    if "nc" not in _STATE:
        _STATE["nc"] = _build_nc()
    nc = _STATE["nc"]

    in_maps = _shard_inputs(**inputs)
    res = run_bass_kernel_spmd(nc, in_maps, core_ids=list(range(NC)))
    r = res.results
    out_s = np.asarray(r[0]["out_s"], dtype=np.float32).reshape(1, 1)
    h_new = np.asarray(r[0]["h_new_out"], dtype=np.float32).reshape(1, 1, H)
    attn = np.concatenate(
        [np.asarray(r[c]["attn_part"], dtype=np.float32) for c in range(NC)]
    ).reshape(1, L)
    return out_s, h_new, attn
